# revision 11
# baseline (speedup 1.0000x reference)
"""Trainium2 Bass kernel for CausalAttentiveStatisticsPooling (v2).

Per batch element b (data-parallel over 8 cores):
  c_mean   = cumsum(x)/count, c_std = sqrt(cumsum(x^2)/count - c_mean^2)
  h        = tanh(w1^T [x; c_mean; c_std] + b1); scores = w2^T h + b2 per key
  attn     = causal softmax -> e_j/E_i with e = exp(scores)*mask, E = cumsum(e)
  w_mean_i = R_i*cumsum(e*x)_i, w_var_i = R_i*cumsum(e*x^2)_i - w_mean_i^2
  out      = [sum_i<L w_mean_i/L, sum_i<L sqrt(w_var_i)/L]

v2 layout/structure:
  - x path is bf16 end-to-end (xT/xN/x2/w1/triangular consts) - full-rate
    matmuls with light quantization (<0.5% rel; tolerance 2e-2).
  - Phase-1 evictions fused: sqm=ACT.Square(s1*rcnt), var1=DVE.STT(s2*rcnt-sqm),
    cs=ACT.Sqrt(var1+eps).  c_mean chunk tiles never materialized.
  - c_std transposed for the MLP with DMA-transposes (no PE transposes, no
    PSUM transpose bank).
  - final_mean needs no per-query w_mean eviction: suffix trick
    G_j = sum_{i>=j} finalw_i*R_i, final_mean = sum_j (e_j*G_j)*x_j.
  - hpre eviction = DVE.STT(ph + b1 + Pm_slice) (kills ident matmuls).
  - clip(var,eps) replaced by sqrt(var + EPSB) with EPSB above the fp32
    cancellation noise floor.
"""

import numpy as np
import ml_dtypes

B, C, T, A = 8, 512, 2048, 128
NCH = T // 128  # 16 T-chunks
NEG = -30000.0
EPSB = 2e-5
EPSC = 1e-12

BF = ml_dtypes.bfloat16

# f32 blob columns
CF_RCNT = 0        # (128,16)
CF_MASKEXP = 16    # (128,16)
CF_B1 = 32         # (128,1)
CF_EPS = 33        # (128,1)
CF_SUTRI16 = 34    # (16,16) rows 0:16
CF_LTRI16 = 50     # (16,16) rows 0:16
CF_ONESC = 66      # (128,1)
NF = 67
# f32r blob columns
CR_TRIL = 0        # (128,128)
CR_FINALW = 128    # (128,16)
CR_ONESC = 144     # (128,1)
NR = 145
# bf16 blob columns
CB_TRIU = 0        # (128,128)
CB_ONESCOLS = 128  # (128,256)
CB_SUTRI48 = 384   # (48,16) rows 0:48
CB_W2 = 400        # (128,1)
CB_ONESC = 401     # (128,1)
NB = 402

_CACHE = {}


def _build():
    import concourse.bass as bass
    import concourse.mybir as mybir
    import concourse.tile as tile
    from concourse.tile import add_dep_helper
    from concourse import bacc

    f32 = mybir.dt.float32
    f32r = mybir.dt.float32r
    bf16 = mybir.dt.bfloat16
    AF = mybir.ActivationFunctionType
    OP = mybir.AluOpType

    nc = bacc.Bacc("TRN2", target_bir_lowering=False, debug=False,
                   num_devices=8)

    def din(name, shape, dt):
        return nc.dram_tensor(name, shape, dt, kind="ExternalInput").ap()

    d_xT = din("xT", (T, C), bf16)
    d_x2 = din("x2T", (T, C), bf16)
    d_xN = din("xN", (C, T), bf16)
    d_w1 = din("w1b", (128, 12 * A), bf16)
    d_cf = din("cstf", (128, NF), f32)
    d_cr = din("cstr", (128, NR), f32r)
    d_cb = din("cstb", (128, NB), bf16)
    d_rb = din("rcntb", (128, T), bf16)
    d_out = nc.dram_tensor("out", (1, 2 * C), f32, kind="ExternalOutput").ap()

    from contextlib import ExitStack
    with tile.TileContext(nc) as tc:
        with ExitStack() as stack:
            def pool(name, bufs, space=None):
                kw = {"space": space} if space else {}
                return stack.enter_context(
                    tc.tile_pool(name=name, bufs=bufs, **kw))
            big = pool("big", 1)
            consts = pool("consts", 1)
            colp = pool("colp", 1)
            sqmp = pool("sqmp", 3)
            v1p = pool("v1p", 3)
            csp = pool("csp", 4)
            natp = pool("natp", 6)
            hp = pool("hp", 4)
            hh = pool("hh", 2)
            wtp = pool("wtp", 16)
            zpp = pool("zpp", 3)
            v2p = pool("v2p", 3)
            wsp = pool("wsp", 3)
            ps_s = pool("ps_s", 2, "PSUM")
            ps_s1 = pool("ps_s1", 2, "PSUM")
            ps_tot = pool("ps_tot", 1, "PSUM")
            ps_ca = pool("ps_ca", 1, "PSUM")
            ps_cb = pool("ps_cb", 1, "PSUM")
            # ---------------- DMAs ----------------
            t_cb = consts.tile([128, NB], bf16)
            nc.sync.dma_start(t_cb, d_cb)
            t_cf = consts.tile([128, NF], f32)
            nc.sync.dma_start(t_cf, d_cf)
            t_xT = big.tile([128, NCH, C], bf16)
            x2 = big.tile([128, NCH, C], bf16)
            d_xT_r = d_xT.rearrange("(n p) c -> p n c", p=128)
            d_x2_r = d_x2.rearrange("(n p) c -> p n c", p=128)
            t_xN = big.tile([128, 4, T], bf16)
            d_xN_r = d_xN.rearrange("(n p) t -> p n t", p=128)
            t_w1 = consts.tile([128, 12, A], bf16)
            for q in range(2):
                nc.gpsimd.dma_start(t_xT[:, 4 * q:4 * (q + 1), :],
                                    d_xT_r[:, 4 * q:4 * (q + 1), :])
                nc.gpsimd.dma_start(x2[:, 4 * q:4 * (q + 1), :],
                                    d_x2_r[:, 4 * q:4 * (q + 1), :])
            nc.gpsimd.dma_start(t_w1, d_w1.rearrange("p (n a) -> p n a", n=12))
            nc.gpsimd.dma_start(t_xN[:, :, 0:1024], d_xN_r[:, :, 0:1024])
            for q in range(2, 4):
                nc.gpsimd.dma_start(t_xT[:, 4 * q:4 * (q + 1), :],
                                    d_xT_r[:, 4 * q:4 * (q + 1), :])
                nc.gpsimd.dma_start(x2[:, 4 * q:4 * (q + 1), :],
                                    d_x2_r[:, 4 * q:4 * (q + 1), :])
            nc.gpsimd.dma_start(t_xN[:, :, 1024:2048], d_xN_r[:, :, 1024:2048])
            t_cr = consts.tile([128, NR], f32r)
            nc.sync.dma_start(t_cr, d_cr)
            rbp = consts.tile([128, 4, 512], bf16)
            nc.gpsimd.dma_start(rbp,
                                d_rb.rearrange("p (g t) -> p g t", g=4))

            # const views
            t_triub = t_cb[:, CB_TRIU:CB_TRIU + 128]
            t_onescols = t_cb[:, CB_ONESCOLS:CB_ONESCOLS + 256]
            t_w2b = t_cb[:, CB_W2:CB_W2 + 1]
            t_onescb = t_cb[:, CB_ONESC:CB_ONESC + 1]
            t_rcnt = t_cf[:, CF_RCNT:CF_RCNT + 16]
            t_maskexp = t_cf[:, CF_MASKEXP:CF_MASKEXP + 16]
            t_b1 = t_cf[:, CF_B1:CF_B1 + 1]
            t_eps = t_cf[:, CF_EPS:CF_EPS + 1]
            t_sutri16 = t_cf[0:16, CF_SUTRI16:CF_SUTRI16 + 16]
            t_ltri16 = t_cf[0:16, CF_LTRI16:CF_LTRI16 + 16]
            t_onescf = t_cf[:, CF_ONESC:CF_ONESC + 1]
            t_tril = t_cr[:, CR_TRIL:CR_TRIL + 128]
            t_finalw = t_cr[:, CR_FINALW:CR_FINALW + 16]
            t_onescr = t_cr[:, CR_ONESC:CR_ONESC + 1]

            def csel_ap(i, k=NCH):
                sl = t_cb[0:k, CB_SUTRI48 + i:CB_SUTRI48 + i + 1]
                return bass.AP(tensor=sl.tensor, offset=sl.offset,
                               ap=[[sl.ap[0][0], k], [0, 128]])

            def bcast16(sb):
                # (16,1) sbuf column -> (16,128) free-broadcast lhsT
                return bass.AP(tensor=sb.tensor, offset=sb.offset,
                               ap=[[sb.ap[0][0], 16], [0, 128]])

            # ---------------- phase-1 totals (two halves) ----------------
            tot1a = colp.tile([8, C], bf16)
            tot2a = colp.tile([8, C], bf16)
            tot1b = colp.tile([16, C], bf16)
            tot2b = colp.tile([16, C], bf16)
            for half in range(2):
                lo, hi = 8 * half, 8 * half + 8
                ps_t1 = ps_tot.tile([16, C], f32, tag="tX")
                ps_t2 = ps_tot.tile([16, C], f32, tag="tY")
                for i in range(lo, hi):
                    oc = t_onescols[:, 16 * i:16 * (i + 1)]
                    nc.tensor.matmul(ps_t1[:], oc, t_xT[:, i, :],
                                     start=(i == lo), stop=(i == hi - 1))
                for i in range(lo, hi):
                    oc = t_onescols[:, 16 * i:16 * (i + 1)]
                    nc.tensor.matmul(ps_t2[:], oc, x2[:, i, :],
                                     start=(i == lo), stop=(i == hi - 1))
                if half == 0:
                    nc.vector.tensor_copy(tot1a[:], ps_t1[0:8, :])
                    nc.vector.tensor_copy(tot2a[:], ps_t2[0:8, :])
                else:
                    nc.vector.tensor_copy(tot1b[:], ps_t1[:])
                    nc.vector.tensor_copy(tot2b[:], ps_t2[:])

            # ---------------- c_mean MLP path: project-then-scan ----------
            Pm_sb = big.tile([128, T], f32r)
            zeros512 = consts.tile([128, 512], f32)
            nc.vector.memset(zeros512[:], 0.0)
            for g in range(4):
                pm_ps = ps_ca.tile([A, 512], f32, tag="cA")
                for cb in range(4):
                    nc.tensor.matmul(pm_ps[:], t_w1[:, 4 + cb, :],
                                     t_xN[:, cb, 512 * g:512 * (g + 1)],
                                     start=(cb == 0), stop=(cb == 3))
                sl = Pm_sb[:, 512 * g:512 * (g + 1)]
                init = (0.0 if g == 0
                        else Pm_sb[:, 512 * g - 1:512 * g].bitcast(f32))
                nc.vector.tensor_tensor_scan(sl, pm_ps[:], zeros512[:],
                                             initial=init,
                                             op0=OP.add, op1=OP.add)
            for g in range(4):
                sl = Pm_sb[:, 512 * g:512 * (g + 1)]
                nc.gpsimd.tensor_mul(sl, sl.bitcast(f32), rbp[:, g, :])

            # ---------------- phase 1 + MLP ----------------
            css = [None] * NCH
            nats = [None] * NCH
            hpres = [None] * 4
            last_cs_inst = None
            for i in range(NCH):
                s1 = ps_s.tile([128, C], f32, tag="sA")
                nc.tensor.matmul(s1[:], t_triub, t_xT[:, i, :],
                                 start=True, stop=(i == 0))
                s2 = ps_s1.tile([128, C], f32, tag="sB")
                nc.tensor.matmul(s2[:], t_triub, x2[:, i, :],
                                 start=True, stop=(i == 0))
                if i > 0:
                    lastA = (i <= 8)
                    nc.tensor.matmul(s1[:], csel_ap(i, 8), tot1a[:],
                                     start=False, stop=lastA)
                    nc.tensor.matmul(s2[:], csel_ap(i, 8), tot2a[:],
                                     start=False, stop=lastA)
                    if i > 8:
                        nc.tensor.matmul(s1[:], csel_ap(i), tot1b[:],
                                         start=False, stop=True)
                        nc.tensor.matmul(s2[:], csel_ap(i), tot2b[:],
                                         start=False, stop=True)
                sqm = sqmp.tile([128, C], f32, tag="sqm")
                nc.scalar.activation(sqm[:], s1[:], AF.Square,
                                     scale=t_rcnt[:, i:i + 1])
                var1p = v1p.tile([128, C], bf16, tag="v1p")
                nc.vector.scalar_tensor_tensor(var1p[:], s2[:],
                                               t_rcnt[:, i:i + 1], sqm[:],
                                               op0=OP.mult, op1=OP.subtract)
                var1 = v1p.tile([128, C], bf16, tag="v1")
                nc.vector.tensor_scalar_max(var1[:], var1p[:], EPSC)
                cs = csp.tile([128, C], bf16, tag="cs")
                cs_inst = nc.scalar.activation(cs[:], var1[:], AF.Sqrt)
                css[i] = cs
                last_cs_inst = cs_inst
                nat = natp.tile([128, 4, 128], bf16, tag="nat")
                nc.sync.dma_start_transpose(nat[:], cs[:])
                nats[i] = nat

                if i % 4 == 3:
                    g = i // 4
                    ph = ps_ca.tile([A, 512], f32, tag="cA")
                    for cb in range(4):
                        nc.tensor.matmul(ph[:], t_w1[:, cb, :],
                                         t_xN[:, cb, 512 * g:512 * (g + 1)],
                                         start=(cb == 0), stop=False)
                    for k in range(4):
                        for cb in range(4):
                            nc.tensor.matmul(
                                ph[:, 128 * k:128 * (k + 1)],
                                t_w1[:, 8 + cb, :],
                                nats[4 * g + k][:, cb, :],
                                start=False, stop=(cb == 3))
                    hpre = hp.tile([A, 512], f32, tag="hpre")
                    nc.vector.scalar_tensor_tensor(
                        hpre[:], ph[:], t_b1,
                        Pm_sb[:, 512 * g:512 * (g + 1)].bitcast(f32),
                        op0=OP.add, op1=OP.add)
                    hpres[g] = hpre

            # ---------------- deferred tanh/scores/e ----------------
            eTf = colp.tile([128, NCH], f32)
            eTb = colp.tile([128, NCH], bf16)
            eR = colp.tile([128, NCH], f32)
            ps_tM = ps_tot.tile([16, C], f32, tag="tX")
            ps_tA = ps_tot.tile([16, C], f32, tag="tY")
            wtris, wcols = [], []
            last_exp_inst = None
            for g in range(4):
                h = hh.tile([A, 512], bf16, tag="h_sb")
                tanh_inst = nc.scalar.activation(h[:], hpres[g][:], AF.Tanh)
                add_dep_helper(tanh_inst.ins, last_cs_inst.ins, sync=False,
                               reason="keep exp-table ACT ops after all sqrts")
                ps_sc = ps_cb.tile([128, 4], f32, tag="cB")
                for k in range(4):
                    nc.tensor.matmul(
                        ps_sc[:, k:k + 1],
                        h[:, 128 * k:128 * (k + 1)],
                        t_w2b, start=True, stop=True)
                nc.vector.tensor_add(eTf[:, 4 * g:4 * g + 4], ps_sc[:],
                                     t_maskexp[:, 4 * g:4 * g + 4])
                last_exp_inst = nc.scalar.activation(
                    eTb[:, 4 * g:4 * g + 4], eTf[:, 4 * g:4 * g + 4], AF.Exp)
                nc.vector.tensor_copy(eR[:, 4 * g:4 * g + 4],
                                      eTb[:, 4 * g:4 * g + 4])
                for ii in range(4 * g, 4 * g + 4):
                    wtri = wtp.tile([128, 128], bf16, tag=f"wtri_{ii}")
                    nc.vector.tensor_scalar_mul(
                        wtri[:], t_triub, eR[:, ii:ii + 1])
                    wcol = wtp.tile([128, 16], bf16, tag=f"wcol_{ii}")
                    nc.vector.tensor_scalar_mul(
                        wcol[:], t_onescols[:, 16 * ii:16 * (ii + 1)],
                        eR[:, ii:ii + 1])
                    wtris.append(wtri)
                    wcols.append(wcol)
                    nc.tensor.matmul(ps_tM[:], wcol[:], t_xT[:, ii, :],
                                     start=(ii == 0), stop=(ii == NCH - 1))
                    nc.tensor.matmul(ps_tA[:], wcol[:], x2[:, ii, :],
                                     start=(ii == 0), stop=(ii == NCH - 1))

            totM = colp.tile([16, C], bf16)
            nc.vector.tensor_copy(totM[:], ps_tM[:])
            totA = colp.tile([16, C], bf16)
            nc.vector.tensor_copy(totA[:], ps_tA[:])

            # ---------------- E -> R, G ----------------
            eE = ps_cb.tile([128, NCH], f32, tag="cB")
            nc.tensor.matmul(eE[:], t_triub, eTb[:], start=True, stop=False)
            ps_et = ps_ca.tile([16, 1], f32, tag="cA")
            nc.tensor.matmul(ps_et[:], eTb[:], t_onescb, start=True, stop=True)
            etot = colp.tile([16, 1], f32)
            nc.vector.tensor_copy(etot[:], ps_et[:])
            nc.tensor.matmul(eE[:], bcast16(etot), t_sutri16, start=False,
                             stop=True)
            R_col = colp.tile([128, NCH], f32)
            nc.vector.reciprocal(R_col[:], eE[:])
            # G = suffix-sum of finalw*R (within chunks via tril, carry via
            # strictly-lower 16x16 over chunk totals)
            RLr = colp.tile([128, NCH], f32r)
            nc.vector.tensor_mul(RLr[:], R_col[:], t_finalw.bitcast(f32))
            ps_G = ps_cb.tile([128, NCH], f32, tag="cB")
            nc.tensor.matmul(ps_G[:], t_tril, RLr[:], start=True, stop=False)
            ps_rt = ps_ca.tile([16, 1], f32, tag="cA")
            nc.tensor.matmul(ps_rt[:], RLr[:].bitcast(f32), t_onescf,
                             start=True, stop=True)
            rtot = colp.tile([16, 1], f32)
            nc.vector.tensor_copy(rtot[:], ps_rt[:])
            nc.tensor.matmul(ps_G[:], bcast16(rtot), t_ltri16, start=False,
                             stop=True)
            wG = colp.tile([128, NCH], bf16)
            nc.vector.tensor_mul(wG[:], eTb[:], ps_G[:])

            # ---------------- phase 2 ----------------
            ps_fm = ps_ca.tile([1, C], f32, tag="cA")
            ps_fs = ps_cb.tile([1, C], f32, tag="cB")
            first_ws_done = False
            for i in range(NCH):
                mp = ps_s.tile([128, C], f32, tag="sA")
                nc.tensor.matmul(mp[:], wtris[i][:], t_xT[:, i, :],
                                 start=True, stop=(i == 0))
                if i > 0:
                    nc.tensor.matmul(mp[:], csel_ap(i), totM[:], start=False,
                                     stop=True)
                ap = ps_s1.tile([128, C], f32, tag="sB")
                nc.tensor.matmul(ap[:], wtris[i][:], x2[:, i, :],
                                 start=True, stop=(i == 0))
                if i > 0:
                    nc.tensor.matmul(ap[:], csel_ap(i), totA[:], start=False,
                                     stop=True)
                zp = zpp.tile([128, C], f32, tag="zp")
                nc.scalar.activation(zp[:], mp[:], AF.Square,
                                     scale=R_col[:, i:i + 1])
                var2p = v2p.tile([128, C], bf16, tag="v2p")
                nc.vector.scalar_tensor_tensor(var2p[:], ap[:],
                                               R_col[:, i:i + 1], zp[:],
                                               op0=OP.mult, op1=OP.subtract)
                var2 = v2p.tile([128, C], bf16, tag="v2")
                nc.vector.tensor_scalar_max(var2[:], var2p[:], EPSC)
                ws = wsp.tile([128, C], f32r, tag="ws")
                ws_inst = nc.scalar.activation(ws[:], var2[:], AF.Sqrt)
                if not first_ws_done:
                    add_dep_helper(ws_inst.ins, last_exp_inst.ins, sync=False,
                                   reason="sqrt-table reload only after exps")
                    first_ws_done = True
                nc.tensor.matmul(ps_fm[:], wG[:, i:i + 1], t_xT[:, i, :],
                                 start=(i == 0), stop=(i == NCH - 1))
                nc.tensor.matmul(ps_fs[:], t_finalw[:, i:i + 1], ws[:],
                                 start=(i == 0), stop=(i == NCH - 1))

            out_sb = colp.tile([1, 2 * C], f32)
            nc.vector.tensor_copy(out_sb[:, 0:C], ps_fm[:])
            nc.vector.tensor_copy(out_sb[:, C:2 * C], ps_fs[:])
            nc.sync.dma_start(d_out, out_sb[:])

    nc.compile()
    return nc


def _host_inputs(x, lengths, w1, b1, w2, b2):
    x = np.asarray(x, np.float32)
    lengths = np.asarray(lengths)
    w1 = np.asarray(w1, np.float32)
    b1 = np.asarray(b1, np.float32)
    w2 = np.asarray(w2, np.float32)
    b2 = np.asarray(b2, np.float32)

    cstf0 = np.zeros((128, NF), np.float32)
    cstf0[:, CF_B1] = b1
    cstf0[:, CF_EPS] = EPSB
    sutri16 = np.triu(np.ones((16, 16), np.float32), 1)
    cstf0[0:16, CF_SUTRI16:CF_SUTRI16 + 16] = sutri16
    cstf0[0:16, CF_LTRI16:CF_LTRI16 + 16] = np.tril(
        np.ones((16, 16), np.float32), -1)
    cstf0[:, CF_ONESC] = 1.0

    cstr = np.zeros((128, NR), np.float32)
    cstr[:, CR_TRIL:CR_TRIL + 128] = np.tril(np.ones((128, 128), np.float32))
    cstr[:, CR_ONESC] = 1.0

    cstb = np.zeros((128, NB), np.float32)
    cstb[:, CB_TRIU:CB_TRIU + 128] = np.triu(np.ones((128, 128), np.float32))
    onescols = np.zeros((128, 256), np.float32)
    for i in range(NCH):
        onescols[:, 16 * i + i] = 1.0
    cstb[:, CB_ONESCOLS:CB_ONESCOLS + 256] = onescols
    sutri48 = np.zeros((48, 16), np.float32)
    sutri48[0:16] = sutri16
    sutri48[32:48] = sutri16
    cstb[0:48, CB_SUTRI48:CB_SUTRI48 + 16] = sutri48
    cstb[:, CB_W2] = w2[:, 0]
    cstb[:, CB_ONESC] = 1.0
    cstb = cstb.astype(BF)

    tt = np.arange(T)
    w1b = np.ascontiguousarray(
        w1.reshape(12, 128, A).transpose(1, 0, 2).reshape(128, 12 * A)
    ).astype(BF)

    maps = []
    for b in range(B):
        L = int(lengths[b])
        rcnt = (1.0 / np.minimum(tt + 1, max(L, 1))).astype(np.float32)
        maskexp = (float(b2[0]) +
                   np.where(tt < L, 0.0, NEG)).astype(np.float32)
        finalw = np.where(tt < L, 1.0 / max(L, 1), 0.0).astype(np.float32)
        cstf = cstf0.copy()
        cstf[:, CF_RCNT:CF_RCNT + 16] = rcnt.reshape(NCH, 128).T
        cstf[:, CF_MASKEXP:CF_MASKEXP + 16] = maskexp.reshape(NCH, 128).T
        cstr_b = cstr.copy()
        cstr_b[:, CR_FINALW:CR_FINALW + 16] = finalw.reshape(NCH, 128).T
        maps.append({
            "xT": np.ascontiguousarray(x[b].T).astype(BF),
            "x2T": np.ascontiguousarray((x[b] ** 2).T).astype(BF),
            "xN": np.ascontiguousarray(x[b]).astype(BF),
            "w1b": w1b,
            "cstf": cstf,
            "cstr": cstr_b,
            "cstb": cstb,
            "rcntb": np.ascontiguousarray(
                np.broadcast_to(rcnt[None, :], (128, T))).astype(BF),
        })
    return maps


def kernel(x, lengths, w1, b1, w2, b2):
    from concourse.bass_utils import run_bass_kernel_spmd

    if "nc" not in _CACHE:
        _CACHE["nc"] = _build()
    nc = _CACHE["nc"]
    maps = _host_inputs(x, lengths, w1, b1, w2, b2)
    res = run_bass_kernel_spmd(nc, maps, list(range(B))).results
    out = np.stack([res[b]["out"][0] for b in range(B)], axis=0)
    return out.astype(np.float32)


# revision 12
# speedup vs baseline: 1.0874x; 1.0874x over previous
"""Trainium2 Bass kernel for CausalAttentiveStatisticsPooling (v2).

Per batch element b (data-parallel over 8 cores):
  c_mean   = cumsum(x)/count, c_std = sqrt(cumsum(x^2)/count - c_mean^2)
  h        = tanh(w1^T [x; c_mean; c_std] + b1); scores = w2^T h + b2 per key
  attn     = causal softmax -> e_j/E_i with e = exp(scores)*mask, E = cumsum(e)
  w_mean_i = R_i*cumsum(e*x)_i, w_var_i = R_i*cumsum(e*x^2)_i - w_mean_i^2
  out      = [sum_i<L w_mean_i/L, sum_i<L sqrt(w_var_i)/L]

v2 layout/structure:
  - x path is bf16 end-to-end (xT/xN/x2/w1/triangular consts) - full-rate
    matmuls with light quantization (<0.5% rel; tolerance 2e-2).
  - Phase-1 evictions fused: sqm=ACT.Square(s1*rcnt), var1=DVE.STT(s2*rcnt-sqm),
    cs=ACT.Sqrt(var1+eps).  c_mean chunk tiles never materialized.
  - c_std transposed for the MLP with DMA-transposes (no PE transposes, no
    PSUM transpose bank).
  - final_mean needs no per-query w_mean eviction: suffix trick
    G_j = sum_{i>=j} finalw_i*R_i, final_mean = sum_j (e_j*G_j)*x_j.
  - hpre eviction = DVE.STT(ph + b1 + Pm_slice) (kills ident matmuls).
  - clip(var,eps) replaced by sqrt(var + EPSB) with EPSB above the fp32
    cancellation noise floor.
"""

import numpy as np
import ml_dtypes

B, C, T, A = 8, 512, 2048, 128
NCH = T // 128  # 16 T-chunks
NEG = -30000.0
EPSB = 2e-5
EPSC = 1e-12

BF = ml_dtypes.bfloat16

# f32 blob columns
CF_RCNT = 0        # (128,16)
CF_MASKEXP = 16    # (128,16)
CF_B1 = 32         # (128,1)
CF_EPS = 33        # (128,1)
CF_SUTRI16 = 34    # (16,16) rows 0:16
CF_LTRI16 = 50     # (16,16) rows 0:16
CF_ONESC = 66      # (128,1)
NF = 67
# f32r blob columns
CR_TRIL = 0        # (128,128)
CR_FINALW = 128    # (128,16)
CR_ONESC = 144     # (128,1)
NR = 145
# bf16 blob columns
CB_TRIU = 0        # (128,128)
CB_ONESCOLS = 128  # (128,256)
CB_SUTRI48 = 384   # (48,16) rows 0:48
CB_W2 = 400        # (128,1)
CB_ONESC = 401     # (128,1)
NB = 402

_CACHE = {}


def _build():
    import concourse.bass as bass
    import concourse.mybir as mybir
    import concourse.tile as tile
    from concourse.tile import add_dep_helper
    from concourse import bacc

    f32 = mybir.dt.float32
    f32r = mybir.dt.float32r
    bf16 = mybir.dt.bfloat16
    AF = mybir.ActivationFunctionType
    OP = mybir.AluOpType

    nc = bacc.Bacc("TRN2", target_bir_lowering=False, debug=False,
                   num_devices=8)

    def din(name, shape, dt):
        return nc.dram_tensor(name, shape, dt, kind="ExternalInput").ap()

    d_xT = din("xT", (T, C), bf16)
    d_x2 = din("x2T", (T, C), bf16)
    d_xN = din("xN", (C, T), bf16)
    d_w1 = din("w1b", (128, 12 * A), bf16)
    d_cf = din("cstf", (128, NF), f32)
    d_cr = din("cstr", (128, NR), f32r)
    d_cb = din("cstb", (128, NB), bf16)
    d_rb = din("rcntb", (128, T), bf16)
    d_out = nc.dram_tensor("out", (1, 2 * C), f32, kind="ExternalOutput").ap()

    from contextlib import ExitStack
    with tile.TileContext(nc) as tc:
        with ExitStack() as stack:
            def pool(name, bufs, space=None):
                kw = {"space": space} if space else {}
                return stack.enter_context(
                    tc.tile_pool(name=name, bufs=bufs, **kw))
            big = pool("big", 1)
            consts = pool("consts", 1)
            colp = pool("colp", 1)
            sqmp = pool("sqmp", 3)
            v1p = pool("v1p", 3)
            csp = pool("csp", 4)
            natp = pool("natp", 6)
            hp = pool("hp", 4)
            hh = pool("hh", 2)
            wtp = pool("wtp", 16)
            zpp = pool("zpp", 3)
            v2p = pool("v2p", 3)
            wsp = pool("wsp", 3)
            ps_s = pool("ps_s", 2, "PSUM")
            ps_s1 = pool("ps_s1", 2, "PSUM")
            ps_tot = pool("ps_tot", 1, "PSUM")
            ps_ca = pool("ps_ca", 1, "PSUM")
            ps_cb = pool("ps_cb", 1, "PSUM")
            # ---------------- DMAs ----------------
            t_cb = consts.tile([128, NB], bf16)
            nc.sync.dma_start(t_cb, d_cb)
            t_cf = consts.tile([128, NF], f32)
            nc.sync.dma_start(t_cf, d_cf)
            t_xT = big.tile([128, NCH, C], bf16)
            x2 = big.tile([128, NCH, C], bf16)
            d_xT_r = d_xT.rearrange("(n p) c -> p n c", p=128)
            d_x2_r = d_x2.rearrange("(n p) c -> p n c", p=128)
            t_xN = big.tile([128, 4, T], bf16)
            d_xN_r = d_xN.rearrange("(n p) t -> p n t", p=128)
            t_w1 = consts.tile([128, 12, A], bf16)
            for q in range(4):
                nc.gpsimd.dma_start(t_xT[:, 4 * q:4 * (q + 1), :],
                                    d_xT_r[:, 4 * q:4 * (q + 1), :])
                nc.gpsimd.dma_start(x2[:, 4 * q:4 * (q + 1), :],
                                    d_x2_r[:, 4 * q:4 * (q + 1), :])
            nc.gpsimd.dma_start(t_w1, d_w1.rearrange("p (n a) -> p n a", n=12))
            nc.gpsimd.dma_start(t_xN[:, :, 0:1024], d_xN_r[:, :, 0:1024])
            nc.gpsimd.dma_start(t_xN[:, :, 1024:2048], d_xN_r[:, :, 1024:2048])
            t_cr = consts.tile([128, NR], f32r)
            nc.sync.dma_start(t_cr, d_cr)
            rbp = consts.tile([128, 4, 512], bf16)
            nc.gpsimd.dma_start(rbp,
                                d_rb.rearrange("p (g t) -> p g t", g=4))

            # const views
            t_triub = t_cb[:, CB_TRIU:CB_TRIU + 128]
            t_onescols = t_cb[:, CB_ONESCOLS:CB_ONESCOLS + 256]
            t_w2b = t_cb[:, CB_W2:CB_W2 + 1]
            t_onescb = t_cb[:, CB_ONESC:CB_ONESC + 1]
            t_rcnt = t_cf[:, CF_RCNT:CF_RCNT + 16]
            t_maskexp = t_cf[:, CF_MASKEXP:CF_MASKEXP + 16]
            t_b1 = t_cf[:, CF_B1:CF_B1 + 1]
            t_eps = t_cf[:, CF_EPS:CF_EPS + 1]
            t_sutri16 = t_cf[0:16, CF_SUTRI16:CF_SUTRI16 + 16]
            t_ltri16 = t_cf[0:16, CF_LTRI16:CF_LTRI16 + 16]
            t_onescf = t_cf[:, CF_ONESC:CF_ONESC + 1]
            t_tril = t_cr[:, CR_TRIL:CR_TRIL + 128]
            t_finalw = t_cr[:, CR_FINALW:CR_FINALW + 16]
            t_onescr = t_cr[:, CR_ONESC:CR_ONESC + 1]

            def csel_ap(i, k=NCH):
                sl = t_cb[0:k, CB_SUTRI48 + i:CB_SUTRI48 + i + 1]
                return bass.AP(tensor=sl.tensor, offset=sl.offset,
                               ap=[[sl.ap[0][0], k], [0, 128]])

            def bcast16(sb):
                # (16,1) sbuf column -> (16,128) free-broadcast lhsT
                return bass.AP(tensor=sb.tensor, offset=sb.offset,
                               ap=[[sb.ap[0][0], 16], [0, 128]])

            # ---------------- phase-1 totals (two halves) ----------------
            tot1a = colp.tile([8, C], bf16)
            tot2a = colp.tile([8, C], bf16)
            tot1b = colp.tile([16, C], bf16)
            tot2b = colp.tile([16, C], bf16)
            for half in range(2):
                lo, hi = 8 * half, 8 * half + 8
                ps_t1 = ps_tot.tile([16, C], f32, tag="tX")
                ps_t2 = ps_tot.tile([16, C], f32, tag="tY")
                for i in range(lo, hi):
                    oc = t_onescols[:, 16 * i:16 * (i + 1)]
                    nc.tensor.matmul(ps_t1[:], oc, t_xT[:, i, :],
                                     start=(i == lo), stop=(i == hi - 1))
                for i in range(lo, hi):
                    oc = t_onescols[:, 16 * i:16 * (i + 1)]
                    nc.tensor.matmul(ps_t2[:], oc, x2[:, i, :],
                                     start=(i == lo), stop=(i == hi - 1))
                if half == 0:
                    nc.vector.tensor_copy(tot1a[:], ps_t1[0:8, :])
                    nc.vector.tensor_copy(tot2a[:], ps_t2[0:8, :])
                else:
                    nc.vector.tensor_copy(tot1b[:], ps_t1[:])
                    nc.vector.tensor_copy(tot2b[:], ps_t2[:])

            # ---------------- c_mean MLP path: project-then-scan ----------
            Pm_sb = big.tile([128, T], f32r)
            zeros512 = consts.tile([128, 512], f32)
            nc.vector.memset(zeros512[:], 0.0)
            for g in range(4):
                pm_ps = ps_ca.tile([A, 512], f32, tag="cA")
                for cb in range(4):
                    nc.tensor.matmul(pm_ps[:], t_w1[:, 4 + cb, :],
                                     t_xN[:, cb, 512 * g:512 * (g + 1)],
                                     start=(cb == 0), stop=(cb == 3))
                sl = Pm_sb[:, 512 * g:512 * (g + 1)]
                init = (0.0 if g == 0
                        else Pm_sb[:, 512 * g - 1:512 * g].bitcast(f32))
                nc.vector.tensor_tensor_scan(sl, pm_ps[:], zeros512[:],
                                             initial=init,
                                             op0=OP.add, op1=OP.add)
            for g in range(4):
                sl = Pm_sb[:, 512 * g:512 * (g + 1)]
                nc.gpsimd.tensor_mul(sl, sl.bitcast(f32), rbp[:, g, :])

            # ---------------- phase 1 + MLP ----------------
            css = [None] * NCH
            nats = [None] * NCH
            hpres = [None] * 4
            last_cs_inst = None
            for i in range(NCH):
                s1 = ps_s.tile([128, C], f32, tag="sA")
                nc.tensor.matmul(s1[:], t_triub, t_xT[:, i, :],
                                 start=True, stop=(i == 0))
                s2 = ps_s1.tile([128, C], f32, tag="sB")
                nc.tensor.matmul(s2[:], t_triub, x2[:, i, :],
                                 start=True, stop=(i == 0))
                if i > 0:
                    lastA = (i <= 8)
                    nc.tensor.matmul(s1[:], csel_ap(i, 8), tot1a[:],
                                     start=False, stop=lastA)
                    nc.tensor.matmul(s2[:], csel_ap(i, 8), tot2a[:],
                                     start=False, stop=lastA)
                    if i > 8:
                        nc.tensor.matmul(s1[:], csel_ap(i), tot1b[:],
                                         start=False, stop=True)
                        nc.tensor.matmul(s2[:], csel_ap(i), tot2b[:],
                                         start=False, stop=True)
                sqm = sqmp.tile([128, C], f32, tag="sqm")
                nc.scalar.activation(sqm[:], s1[:], AF.Square,
                                     scale=t_rcnt[:, i:i + 1])
                var1p = v1p.tile([128, C], bf16, tag="v1p")
                nc.vector.scalar_tensor_tensor(var1p[:], s2[:],
                                               t_rcnt[:, i:i + 1], sqm[:],
                                               op0=OP.mult, op1=OP.subtract)
                var1 = v1p.tile([128, C], bf16, tag="v1")
                nc.vector.tensor_scalar_max(var1[:], var1p[:], EPSC)
                cs = csp.tile([128, C], bf16, tag="cs")
                cs_inst = nc.scalar.activation(cs[:], var1[:], AF.Sqrt)
                css[i] = cs
                last_cs_inst = cs_inst
                nat = natp.tile([128, 4, 128], bf16, tag="nat")
                nc.sync.dma_start_transpose(nat[:], cs[:])
                nats[i] = nat

                if i % 4 == 3:
                    g = i // 4
                    ph = ps_ca.tile([A, 512], f32, tag="cA")
                    for cb in range(4):
                        nc.tensor.matmul(ph[:], t_w1[:, cb, :],
                                         t_xN[:, cb, 512 * g:512 * (g + 1)],
                                         start=(cb == 0), stop=False)
                    for k in range(4):
                        for cb in range(4):
                            nc.tensor.matmul(
                                ph[:, 128 * k:128 * (k + 1)],
                                t_w1[:, 8 + cb, :],
                                nats[4 * g + k][:, cb, :],
                                start=False, stop=(cb == 3))
                    hpre = hp.tile([A, 512], f32, tag="hpre")
                    nc.vector.scalar_tensor_tensor(
                        hpre[:], ph[:], t_b1,
                        Pm_sb[:, 512 * g:512 * (g + 1)].bitcast(f32),
                        op0=OP.add, op1=OP.add)
                    hpres[g] = hpre

            # ---------------- deferred tanh/scores/e ----------------
            eTf = colp.tile([128, NCH], f32)
            eTb = colp.tile([128, NCH], bf16)
            eR = colp.tile([128, NCH], f32)
            ps_tM = ps_tot.tile([16, C], f32, tag="tX")
            ps_tA = ps_tot.tile([16, C], f32, tag="tY")
            wtris, wcols = [], []
            last_exp_inst = None
            for g in range(4):
                h = hh.tile([A, 512], bf16, tag="h_sb")
                tanh_inst = nc.scalar.activation(h[:], hpres[g][:], AF.Tanh)
                add_dep_helper(tanh_inst.ins, last_cs_inst.ins, sync=False,
                               reason="keep exp-table ACT ops after all sqrts")
                ps_sc = ps_cb.tile([128, 4], f32, tag="cB")
                for k in range(4):
                    nc.tensor.matmul(
                        ps_sc[:, k:k + 1],
                        h[:, 128 * k:128 * (k + 1)],
                        t_w2b, start=True, stop=True)
                nc.vector.tensor_add(eTf[:, 4 * g:4 * g + 4], ps_sc[:],
                                     t_maskexp[:, 4 * g:4 * g + 4])
                last_exp_inst = nc.scalar.activation(
                    eTb[:, 4 * g:4 * g + 4], eTf[:, 4 * g:4 * g + 4], AF.Exp)
                nc.vector.tensor_copy(eR[:, 4 * g:4 * g + 4],
                                      eTb[:, 4 * g:4 * g + 4])
                for ii in range(4 * g, 4 * g + 4):
                    wtri = wtp.tile([128, 128], bf16, tag=f"wtri_{ii}")
                    nc.vector.tensor_scalar_mul(
                        wtri[:], t_triub, eR[:, ii:ii + 1])
                    wcol = wtp.tile([128, 16], bf16, tag=f"wcol_{ii}")
                    nc.vector.tensor_scalar_mul(
                        wcol[:], t_onescols[:, 16 * ii:16 * (ii + 1)],
                        eR[:, ii:ii + 1])
                    wtris.append(wtri)
                    wcols.append(wcol)
                    nc.tensor.matmul(ps_tM[:], wcol[:], t_xT[:, ii, :],
                                     start=(ii == 0), stop=(ii == NCH - 1))
                    nc.tensor.matmul(ps_tA[:], wcol[:], x2[:, ii, :],
                                     start=(ii == 0), stop=(ii == NCH - 1))

            totM = colp.tile([16, C], bf16)
            nc.vector.tensor_copy(totM[:], ps_tM[:])
            totA = colp.tile([16, C], bf16)
            nc.vector.tensor_copy(totA[:], ps_tA[:])

            # ---------------- E -> R, G ----------------
            eE = ps_cb.tile([128, NCH], f32, tag="cB")
            nc.tensor.matmul(eE[:], t_triub, eTb[:], start=True, stop=False)
            ps_et = ps_ca.tile([16, 1], f32, tag="cA")
            nc.tensor.matmul(ps_et[:], eTb[:], t_onescb, start=True, stop=True)
            etot = colp.tile([16, 1], f32)
            nc.vector.tensor_copy(etot[:], ps_et[:])
            nc.tensor.matmul(eE[:], bcast16(etot), t_sutri16, start=False,
                             stop=True)
            R_col = colp.tile([128, NCH], f32)
            nc.vector.reciprocal(R_col[:], eE[:])
            # G = suffix-sum of finalw*R (within chunks via tril, carry via
            # strictly-lower 16x16 over chunk totals)
            RLr = colp.tile([128, NCH], f32r)
            nc.vector.tensor_mul(RLr[:], R_col[:], t_finalw.bitcast(f32))
            ps_G = ps_cb.tile([128, NCH], f32, tag="cB")
            nc.tensor.matmul(ps_G[:], t_tril, RLr[:], start=True, stop=False)
            ps_rt = ps_ca.tile([16, 1], f32, tag="cA")
            nc.tensor.matmul(ps_rt[:], RLr[:].bitcast(f32), t_onescf,
                             start=True, stop=True)
            rtot = colp.tile([16, 1], f32)
            nc.vector.tensor_copy(rtot[:], ps_rt[:])
            nc.tensor.matmul(ps_G[:], bcast16(rtot), t_ltri16, start=False,
                             stop=True)
            wG = colp.tile([128, NCH], bf16)
            nc.vector.tensor_mul(wG[:], eTb[:], ps_G[:])

            # ---------------- phase 2 ----------------
            ps_fm = ps_ca.tile([1, C], f32, tag="cA")
            ps_fs = ps_cb.tile([1, C], f32, tag="cB")
            first_ws_done = False
            for i in range(NCH):
                mp = ps_s.tile([128, C], f32, tag="sA")
                nc.tensor.matmul(mp[:], wtris[i][:], t_xT[:, i, :],
                                 start=True, stop=(i == 0))
                if i > 0:
                    nc.tensor.matmul(mp[:], csel_ap(i), totM[:], start=False,
                                     stop=True)
                ap = ps_s1.tile([128, C], f32, tag="sB")
                nc.tensor.matmul(ap[:], wtris[i][:], x2[:, i, :],
                                 start=True, stop=(i == 0))
                if i > 0:
                    nc.tensor.matmul(ap[:], csel_ap(i), totA[:], start=False,
                                     stop=True)
                zp = zpp.tile([128, C], f32, tag="zp")
                nc.scalar.activation(zp[:], mp[:], AF.Square,
                                     scale=R_col[:, i:i + 1])
                var2p = v2p.tile([128, C], bf16, tag="v2p")
                nc.vector.scalar_tensor_tensor(var2p[:], ap[:],
                                               R_col[:, i:i + 1], zp[:],
                                               op0=OP.mult, op1=OP.subtract)
                var2 = v2p.tile([128, C], bf16, tag="v2")
                nc.vector.tensor_scalar_max(var2[:], var2p[:], EPSC)
                ws = wsp.tile([128, C], f32r, tag="ws")
                ws_inst = nc.scalar.activation(ws[:], var2[:], AF.Sqrt)
                if not first_ws_done:
                    add_dep_helper(ws_inst.ins, last_exp_inst.ins, sync=False,
                                   reason="sqrt-table reload only after exps")
                    first_ws_done = True
                nc.tensor.matmul(ps_fm[:], wG[:, i:i + 1], t_xT[:, i, :],
                                 start=(i == 0), stop=(i == NCH - 1))
                nc.tensor.matmul(ps_fs[:], t_finalw[:, i:i + 1], ws[:],
                                 start=(i == 0), stop=(i == NCH - 1))

            out_sb = colp.tile([1, 2 * C], f32)
            nc.vector.tensor_copy(out_sb[:, 0:C], ps_fm[:])
            nc.vector.tensor_copy(out_sb[:, C:2 * C], ps_fs[:])
            nc.sync.dma_start(d_out, out_sb[:])

    nc.compile()
    return nc


def _host_inputs(x, lengths, w1, b1, w2, b2):
    x = np.asarray(x, np.float32)
    lengths = np.asarray(lengths)
    w1 = np.asarray(w1, np.float32)
    b1 = np.asarray(b1, np.float32)
    w2 = np.asarray(w2, np.float32)
    b2 = np.asarray(b2, np.float32)

    cstf0 = np.zeros((128, NF), np.float32)
    cstf0[:, CF_B1] = b1
    cstf0[:, CF_EPS] = EPSB
    sutri16 = np.triu(np.ones((16, 16), np.float32), 1)
    cstf0[0:16, CF_SUTRI16:CF_SUTRI16 + 16] = sutri16
    cstf0[0:16, CF_LTRI16:CF_LTRI16 + 16] = np.tril(
        np.ones((16, 16), np.float32), -1)
    cstf0[:, CF_ONESC] = 1.0

    cstr = np.zeros((128, NR), np.float32)
    cstr[:, CR_TRIL:CR_TRIL + 128] = np.tril(np.ones((128, 128), np.float32))
    cstr[:, CR_ONESC] = 1.0

    cstb = np.zeros((128, NB), np.float32)
    cstb[:, CB_TRIU:CB_TRIU + 128] = np.triu(np.ones((128, 128), np.float32))
    onescols = np.zeros((128, 256), np.float32)
    for i in range(NCH):
        onescols[:, 16 * i + i] = 1.0
    cstb[:, CB_ONESCOLS:CB_ONESCOLS + 256] = onescols
    sutri48 = np.zeros((48, 16), np.float32)
    sutri48[0:16] = sutri16
    sutri48[32:48] = sutri16
    cstb[0:48, CB_SUTRI48:CB_SUTRI48 + 16] = sutri48
    cstb[:, CB_W2] = w2[:, 0]
    cstb[:, CB_ONESC] = 1.0
    cstb = cstb.astype(BF)

    tt = np.arange(T)
    w1b = np.ascontiguousarray(
        w1.reshape(12, 128, A).transpose(1, 0, 2).reshape(128, 12 * A)
    ).astype(BF)

    maps = []
    for b in range(B):
        L = int(lengths[b])
        rcnt = (1.0 / np.minimum(tt + 1, max(L, 1))).astype(np.float32)
        maskexp = (float(b2[0]) +
                   np.where(tt < L, 0.0, NEG)).astype(np.float32)
        finalw = np.where(tt < L, 1.0 / max(L, 1), 0.0).astype(np.float32)
        cstf = cstf0.copy()
        cstf[:, CF_RCNT:CF_RCNT + 16] = rcnt.reshape(NCH, 128).T
        cstf[:, CF_MASKEXP:CF_MASKEXP + 16] = maskexp.reshape(NCH, 128).T
        cstr_b = cstr.copy()
        cstr_b[:, CR_FINALW:CR_FINALW + 16] = finalw.reshape(NCH, 128).T
        maps.append({
            "xT": np.ascontiguousarray(x[b].T).astype(BF),
            "x2T": np.ascontiguousarray((x[b] ** 2).T).astype(BF),
            "xN": np.ascontiguousarray(x[b]).astype(BF),
            "w1b": w1b,
            "cstf": cstf,
            "cstr": cstr_b,
            "cstb": cstb,
            "rcntb": np.ascontiguousarray(
                np.broadcast_to(rcnt[None, :], (128, T))).astype(BF),
        })
    return maps


def kernel(x, lengths, w1, b1, w2, b2):
    from concourse.bass_utils import run_bass_kernel_spmd

    if "nc" not in _CACHE:
        _CACHE["nc"] = _build()
    nc = _CACHE["nc"]
    maps = _host_inputs(x, lengths, w1, b1, w2, b2)
    res = run_bass_kernel_spmd(nc, maps, list(range(B))).results
    out = np.stack([res[b]["out"][0] for b in range(B)], axis=0)
    return out.astype(np.float32)


# revision 13
# speedup vs baseline: 1.1577x; 1.0647x over previous
"""Trainium2 Bass kernel for CausalAttentiveStatisticsPooling (v2).

Per batch element b (data-parallel over 8 cores):
  c_mean   = cumsum(x)/count, c_std = sqrt(cumsum(x^2)/count - c_mean^2)
  h        = tanh(w1^T [x; c_mean; c_std] + b1); scores = w2^T h + b2 per key
  attn     = causal softmax -> e_j/E_i with e = exp(scores)*mask, E = cumsum(e)
  w_mean_i = R_i*cumsum(e*x)_i, w_var_i = R_i*cumsum(e*x^2)_i - w_mean_i^2
  out      = [sum_i<L w_mean_i/L, sum_i<L sqrt(w_var_i)/L]

v2 layout/structure:
  - x path is bf16 end-to-end (xT/xN/x2/w1/triangular consts) - full-rate
    matmuls with light quantization (<0.5% rel; tolerance 2e-2).
  - Phase-1 evictions fused: sqm=ACT.Square(s1*rcnt), var1=DVE.STT(s2*rcnt-sqm),
    cs=ACT.Sqrt(var1+eps).  c_mean chunk tiles never materialized.
  - c_std transposed for the MLP with DMA-transposes (no PE transposes, no
    PSUM transpose bank).
  - final_mean needs no per-query w_mean eviction: suffix trick
    G_j = sum_{i>=j} finalw_i*R_i, final_mean = sum_j (e_j*G_j)*x_j.
  - hpre eviction = DVE.STT(ph + b1 + Pm_slice) (kills ident matmuls).
  - clip(var,eps) replaced by sqrt(var + EPSB) with EPSB above the fp32
    cancellation noise floor.
"""

import numpy as np
import ml_dtypes

B, C, T, A = 8, 512, 2048, 128
NCH = T // 128  # 16 T-chunks
NEG = -30000.0
EPSB = 2e-5
EPSC = 1e-12

BF = ml_dtypes.bfloat16

# f32 blob columns
CF_RCNT = 0        # (128,16)
CF_MASKEXP = 16    # (128,16)
CF_B1 = 32         # (128,1)
CF_EPS = 33        # (128,1)
CF_SUTRI16 = 34    # (16,16) rows 0:16
CF_LTRI16 = 50     # (16,16) rows 0:16
CF_ONESC = 66      # (128,1)
NF = 67
# f32r blob columns
CR_TRIL = 0        # (128,128)
CR_FINALW = 128    # (128,16)
CR_ONESC = 144     # (128,1)
NR = 145
# bf16 blob columns
CB_TRIU = 0        # (128,128)
CB_ONESCOLS = 128  # (128,256)
CB_SUTRI48 = 384   # (48,16) rows 0:48
CB_W2 = 400        # (128,1)
CB_ONESC = 401     # (128,1)
CB_SEL8 = 402      # (8,16) rows 0:8
NB = 418

_CACHE = {}


def _build():
    import concourse.bass as bass
    import concourse.mybir as mybir
    import concourse.tile as tile
    from concourse.tile import add_dep_helper
    from concourse import bacc

    f32 = mybir.dt.float32
    f32r = mybir.dt.float32r
    bf16 = mybir.dt.bfloat16
    AF = mybir.ActivationFunctionType
    OP = mybir.AluOpType

    nc = bacc.Bacc("TRN2", target_bir_lowering=False, debug=False,
                   num_devices=8)

    def din(name, shape, dt):
        return nc.dram_tensor(name, shape, dt, kind="ExternalInput").ap()

    d_xT = din("xT", (T, C), bf16)
    d_x2 = din("x2T", (T, C), bf16)
    d_xN = din("xN", (C, T), bf16)
    d_w1 = din("w1b", (128, 12 * A), bf16)
    d_cf = din("cstf", (128, NF), f32)
    d_cr = din("cstr", (128, NR), f32r)
    d_cb = din("cstb", (128, NB), bf16)
    d_rb = din("rcntb", (128, T), bf16)
    d_out = nc.dram_tensor("out", (1, 2 * C), f32, kind="ExternalOutput").ap()

    from contextlib import ExitStack
    with tile.TileContext(nc) as tc:
        with ExitStack() as stack:
            def pool(name, bufs, space=None):
                kw = {"space": space} if space else {}
                return stack.enter_context(
                    tc.tile_pool(name=name, bufs=bufs, **kw))
            big = pool("big", 1)
            consts = pool("consts", 1)
            colp = pool("colp", 1)
            sqmp = pool("sqmp", 3)
            v1p = pool("v1p", 3)
            csp = pool("csp", 4)
            natp = pool("natp", 6)
            hp = pool("hp", 4)
            hh = pool("hh", 2)
            wtp = pool("wtp", 16)
            zpp = pool("zpp", 3)
            v2p = pool("v2p", 3)
            wsp = pool("wsp", 3)
            ps_s = pool("ps_s", 2, "PSUM")
            ps_s1 = pool("ps_s1", 2, "PSUM")
            ps_tot = pool("ps_tot", 1, "PSUM")
            ps_ca = pool("ps_ca", 1, "PSUM")
            ps_cb = pool("ps_cb", 1, "PSUM")
            # ---------------- DMAs ----------------
            t_cb = consts.tile([128, NB], bf16)
            nc.sync.dma_start(t_cb, d_cb)
            t_cf = consts.tile([128, NF], f32)
            nc.sync.dma_start(t_cf, d_cf)
            t_xT = big.tile([128, NCH, C], bf16)
            x2 = big.tile([128, NCH, C], bf16)
            d_xT_r = d_xT.rearrange("(n p) c -> p n c", p=128)
            d_x2_r = d_x2.rearrange("(n p) c -> p n c", p=128)
            t_xN = big.tile([128, 4, T], bf16)
            d_xN_r = d_xN.rearrange("(n p) t -> p n t", p=128)
            t_w1 = consts.tile([128, 12, A], bf16)
            for q in range(8):
                eng = nc.sync if q % 2 == 0 else nc.gpsimd
                eng.dma_start(t_xT[:, 2 * q:2 * (q + 1), :],
                              d_xT_r[:, 2 * q:2 * (q + 1), :])
                eng.dma_start(x2[:, 2 * q:2 * (q + 1), :],
                              d_x2_r[:, 2 * q:2 * (q + 1), :])
            nc.gpsimd.dma_start(t_w1, d_w1.rearrange("p (n a) -> p n a", n=12))
            nc.gpsimd.dma_start(t_xN[:, :, 0:1024], d_xN_r[:, :, 0:1024])
            nc.gpsimd.dma_start(t_xN[:, :, 1024:2048], d_xN_r[:, :, 1024:2048])
            t_cr = consts.tile([128, NR], f32r)
            nc.sync.dma_start(t_cr, d_cr)
            rbp = consts.tile([128, 4, 512], bf16)
            nc.gpsimd.dma_start(rbp,
                                d_rb.rearrange("p (g t) -> p g t", g=4))

            # const views
            t_triub = t_cb[:, CB_TRIU:CB_TRIU + 128]
            t_onescols = t_cb[:, CB_ONESCOLS:CB_ONESCOLS + 256]
            t_w2b = t_cb[:, CB_W2:CB_W2 + 1]
            t_onescb = t_cb[:, CB_ONESC:CB_ONESC + 1]
            t_sel8 = t_cb[0:8, CB_SEL8:CB_SEL8 + 16]
            t_rcnt = t_cf[:, CF_RCNT:CF_RCNT + 16]
            t_maskexp = t_cf[:, CF_MASKEXP:CF_MASKEXP + 16]
            t_b1 = t_cf[:, CF_B1:CF_B1 + 1]
            t_eps = t_cf[:, CF_EPS:CF_EPS + 1]
            t_sutri16 = t_cf[0:16, CF_SUTRI16:CF_SUTRI16 + 16]
            t_ltri16 = t_cf[0:16, CF_LTRI16:CF_LTRI16 + 16]
            t_onescf = t_cf[:, CF_ONESC:CF_ONESC + 1]
            t_tril = t_cr[:, CR_TRIL:CR_TRIL + 128]
            t_finalw = t_cr[:, CR_FINALW:CR_FINALW + 16]
            t_onescr = t_cr[:, CR_ONESC:CR_ONESC + 1]

            def csel_ap(i, k=NCH):
                sl = t_cb[0:k, CB_SUTRI48 + i:CB_SUTRI48 + i + 1]
                return bass.AP(tensor=sl.tensor, offset=sl.offset,
                               ap=[[sl.ap[0][0], k], [0, 128]])

            def bcast16(sb):
                # (16,1) sbuf column -> (16,128) free-broadcast lhsT
                return bass.AP(tensor=sb.tensor, offset=sb.offset,
                               ap=[[sb.ap[0][0], 16], [0, 128]])

            # ---------------- phase-1 totals (two halves) ----------------
            tot1a = colp.tile([8, C], bf16)
            tot2a = colp.tile([8, C], bf16)
            tot1b = colp.tile([16, C], bf16)
            tot2b = colp.tile([16, C], bf16)
            for half in range(2):
                lo, hi = 8 * half, 8 * half + 8
                ps_t1 = ps_tot.tile([16, C], f32, tag="tX")
                ps_t2 = ps_tot.tile([16, C], f32, tag="tY")
                for i in range(lo, hi):
                    oc = t_onescols[:, 16 * i:16 * (i + 1)]
                    nc.tensor.matmul(ps_t1[:], oc, t_xT[:, i, :],
                                     start=(i == lo),
                                     stop=(half == 0 and i == hi - 1))
                for i in range(lo, hi):
                    oc = t_onescols[:, 16 * i:16 * (i + 1)]
                    nc.tensor.matmul(ps_t2[:], oc, x2[:, i, :],
                                     start=(i == lo),
                                     stop=(half == 0 and i == hi - 1))
                if half == 0:
                    nc.vector.tensor_copy(tot1a[:], ps_t1[0:8, :])
                    nc.vector.tensor_copy(tot2a[:], ps_t2[0:8, :])
                else:
                    nc.tensor.matmul(ps_t1[:], t_sel8, tot1a[:],
                                     start=False, stop=True)
                    nc.tensor.matmul(ps_t2[:], t_sel8, tot2a[:],
                                     start=False, stop=True)
                    nc.vector.tensor_copy(tot1b[:], ps_t1[:])
                    nc.vector.tensor_copy(tot2b[:], ps_t2[:])

            # ---------------- c_mean MLP path: project-then-scan ----------
            Pm_sb = big.tile([128, T], f32r)
            zeros512 = consts.tile([128, 512], f32)
            nc.vector.memset(zeros512[:], 0.0)
            for g in range(4):
                pm_ps = ps_ca.tile([A, 512], f32, tag="cA")
                for cb in range(4):
                    nc.tensor.matmul(pm_ps[:], t_w1[:, 4 + cb, :],
                                     t_xN[:, cb, 512 * g:512 * (g + 1)],
                                     start=(cb == 0), stop=(cb == 3))
                sl = Pm_sb[:, 512 * g:512 * (g + 1)]
                init = (0.0 if g == 0
                        else Pm_sb[:, 512 * g - 1:512 * g].bitcast(f32))
                nc.vector.tensor_tensor_scan(sl, pm_ps[:], zeros512[:],
                                             initial=init,
                                             op0=OP.add, op1=OP.add)
            for g in range(4):
                sl = Pm_sb[:, 512 * g:512 * (g + 1)]
                nc.gpsimd.tensor_mul(sl, sl.bitcast(f32), rbp[:, g, :])

            # ---------------- phase 1 + MLP ----------------
            css = [None] * NCH
            nats = [None] * NCH
            hpres = [None] * 4
            last_cs_inst = None
            for i in range(NCH):
                s1 = ps_s.tile([128, C], f32, tag="sA")
                nc.tensor.matmul(s1[:], t_triub, t_xT[:, i, :],
                                 start=True, stop=(i == 0))
                s2 = ps_s1.tile([128, C], f32, tag="sB")
                nc.tensor.matmul(s2[:], t_triub, x2[:, i, :],
                                 start=True, stop=(i == 0))
                if 0 < i <= 8:
                    nc.tensor.matmul(s1[:], csel_ap(i, 8), tot1a[:],
                                     start=False, stop=True)
                    nc.tensor.matmul(s2[:], csel_ap(i, 8), tot2a[:],
                                     start=False, stop=True)
                elif i > 8:
                    nc.tensor.matmul(s1[:], csel_ap(i), tot1b[:],
                                     start=False, stop=True)
                    nc.tensor.matmul(s2[:], csel_ap(i), tot2b[:],
                                     start=False, stop=True)
                sqm = sqmp.tile([128, C], f32, tag="sqm")
                nc.scalar.activation(sqm[:], s1[:], AF.Square,
                                     scale=t_rcnt[:, i:i + 1])
                var1p = v1p.tile([128, C], bf16, tag="v1p")
                nc.vector.scalar_tensor_tensor(var1p[:], s2[:],
                                               t_rcnt[:, i:i + 1], sqm[:],
                                               op0=OP.mult, op1=OP.subtract)
                var1 = v1p.tile([128, C], bf16, tag="v1")
                nc.vector.tensor_scalar_max(var1[:], var1p[:], EPSC)
                cs = csp.tile([128, C], bf16, tag="cs")
                cs_inst = nc.scalar.activation(cs[:], var1[:], AF.Sqrt)
                css[i] = cs
                last_cs_inst = cs_inst
                nat = natp.tile([128, 4, 128], bf16, tag="nat")
                nc.sync.dma_start_transpose(nat[:], cs[:])
                nats[i] = nat

                if i % 4 == 3:
                    g = i // 4
                    ph = ps_ca.tile([A, 512], f32, tag="cA")
                    for cb in range(4):
                        nc.tensor.matmul(ph[:], t_w1[:, cb, :],
                                         t_xN[:, cb, 512 * g:512 * (g + 1)],
                                         start=(cb == 0), stop=False)
                    for k in range(4):
                        for cb in range(4):
                            nc.tensor.matmul(
                                ph[:, 128 * k:128 * (k + 1)],
                                t_w1[:, 8 + cb, :],
                                nats[4 * g + k][:, cb, :],
                                start=False, stop=(cb == 3))
                    hpre = hp.tile([A, 512], f32, tag="hpre")
                    nc.vector.scalar_tensor_tensor(
                        hpre[:], ph[:], t_b1,
                        Pm_sb[:, 512 * g:512 * (g + 1)].bitcast(f32),
                        op0=OP.add, op1=OP.add)
                    hpres[g] = hpre

            # ---------------- deferred tanh/scores/e ----------------
            eTf = colp.tile([128, NCH], f32)
            eTb = colp.tile([128, NCH], bf16)
            eR = colp.tile([128, NCH], f32)
            ps_tM = ps_tot.tile([16, C], f32, tag="tX")
            ps_tA = ps_tot.tile([16, C], f32, tag="tY")
            wtris, wcols = [], []
            last_exp_inst = None
            for g in range(4):
                h = hh.tile([A, 512], bf16, tag="h_sb")
                tanh_inst = nc.scalar.activation(h[:], hpres[g][:], AF.Tanh)
                add_dep_helper(tanh_inst.ins, last_cs_inst.ins, sync=False,
                               reason="keep exp-table ACT ops after all sqrts")
                ps_sc = ps_cb.tile([128, 4], f32, tag="cB")
                for k in range(4):
                    nc.tensor.matmul(
                        ps_sc[:, k:k + 1],
                        h[:, 128 * k:128 * (k + 1)],
                        t_w2b, start=True, stop=True)
                nc.vector.tensor_add(eTf[:, 4 * g:4 * g + 4], ps_sc[:],
                                     t_maskexp[:, 4 * g:4 * g + 4])
                last_exp_inst = nc.scalar.activation(
                    eTb[:, 4 * g:4 * g + 4], eTf[:, 4 * g:4 * g + 4], AF.Exp)
                nc.vector.tensor_copy(eR[:, 4 * g:4 * g + 4],
                                      eTb[:, 4 * g:4 * g + 4])
                for ii in range(4 * g, 4 * g + 4):
                    wtri = wtp.tile([128, 128], bf16, tag=f"wtri_{ii}")
                    nc.vector.tensor_scalar_mul(
                        wtri[:], t_triub, eR[:, ii:ii + 1])
                    wcol = wtp.tile([128, 16], bf16, tag=f"wcol_{ii}")
                    nc.vector.tensor_scalar_mul(
                        wcol[:], t_onescols[:, 16 * ii:16 * (ii + 1)],
                        eR[:, ii:ii + 1])
                    wtris.append(wtri)
                    wcols.append(wcol)
                    nc.tensor.matmul(ps_tM[:], wcol[:], t_xT[:, ii, :],
                                     start=(ii == 0), stop=(ii == NCH - 1))
                    nc.tensor.matmul(ps_tA[:], wcol[:], x2[:, ii, :],
                                     start=(ii == 0), stop=(ii == NCH - 1))

            totM = colp.tile([16, C], bf16)
            nc.vector.tensor_copy(totM[:], ps_tM[:])
            totA = colp.tile([16, C], bf16)
            nc.vector.tensor_copy(totA[:], ps_tA[:])

            # ---------------- E -> R, G ----------------
            eE = ps_cb.tile([128, NCH], f32, tag="cB")
            nc.tensor.matmul(eE[:], t_triub, eTb[:], start=True, stop=False)
            ps_et = ps_ca.tile([16, 1], f32, tag="cA")
            nc.tensor.matmul(ps_et[:], eTb[:], t_onescb, start=True, stop=True)
            etot = colp.tile([16, 1], f32)
            nc.vector.tensor_copy(etot[:], ps_et[:])
            nc.tensor.matmul(eE[:], bcast16(etot), t_sutri16, start=False,
                             stop=True)
            R_col = colp.tile([128, NCH], f32)
            nc.vector.reciprocal(R_col[:], eE[:])
            # G = suffix-sum of finalw*R (within chunks via tril, carry via
            # strictly-lower 16x16 over chunk totals)
            RLr = colp.tile([128, NCH], f32r)
            nc.vector.tensor_mul(RLr[:], R_col[:], t_finalw.bitcast(f32))
            ps_G = ps_cb.tile([128, NCH], f32, tag="cB")
            nc.tensor.matmul(ps_G[:], t_tril, RLr[:], start=True, stop=False)
            ps_rt = ps_ca.tile([16, 1], f32, tag="cA")
            nc.tensor.matmul(ps_rt[:], RLr[:].bitcast(f32), t_onescf,
                             start=True, stop=True)
            rtot = colp.tile([16, 1], f32)
            nc.vector.tensor_copy(rtot[:], ps_rt[:])
            nc.tensor.matmul(ps_G[:], bcast16(rtot), t_ltri16, start=False,
                             stop=True)
            wG = colp.tile([128, NCH], bf16)
            nc.vector.tensor_mul(wG[:], eTb[:], ps_G[:])

            # ---------------- phase 2 ----------------
            ps_fm = ps_ca.tile([1, C], f32, tag="cA")
            ps_fs = ps_cb.tile([1, C], f32, tag="cB")
            first_ws_done = False
            for i in range(NCH):
                mp = ps_s.tile([128, C], f32, tag="sA")
                nc.tensor.matmul(mp[:], wtris[i][:], t_xT[:, i, :],
                                 start=True, stop=(i == 0))
                if i > 0:
                    nc.tensor.matmul(mp[:], csel_ap(i), totM[:], start=False,
                                     stop=True)
                ap = ps_s1.tile([128, C], f32, tag="sB")
                nc.tensor.matmul(ap[:], wtris[i][:], x2[:, i, :],
                                 start=True, stop=(i == 0))
                if i > 0:
                    nc.tensor.matmul(ap[:], csel_ap(i), totA[:], start=False,
                                     stop=True)
                zp = zpp.tile([128, C], f32, tag="zp")
                nc.scalar.activation(zp[:], mp[:], AF.Square,
                                     scale=R_col[:, i:i + 1])
                var2p = v2p.tile([128, C], bf16, tag="v2p")
                nc.vector.scalar_tensor_tensor(var2p[:], ap[:],
                                               R_col[:, i:i + 1], zp[:],
                                               op0=OP.mult, op1=OP.subtract)
                var2 = v2p.tile([128, C], bf16, tag="v2")
                nc.vector.tensor_scalar_max(var2[:], var2p[:], EPSC)
                ws = wsp.tile([128, C], f32r, tag="ws")
                ws_inst = nc.scalar.activation(ws[:], var2[:], AF.Sqrt)
                if not first_ws_done:
                    add_dep_helper(ws_inst.ins, last_exp_inst.ins, sync=False,
                                   reason="sqrt-table reload only after exps")
                    first_ws_done = True
                nc.tensor.matmul(ps_fm[:], wG[:, i:i + 1], t_xT[:, i, :],
                                 start=(i == 0), stop=(i == NCH - 1))
                nc.tensor.matmul(ps_fs[:], t_finalw[:, i:i + 1], ws[:],
                                 start=(i == 0), stop=(i == NCH - 1))

            out_sb = colp.tile([1, 2 * C], f32)
            nc.vector.tensor_copy(out_sb[:, 0:C], ps_fm[:])
            nc.vector.tensor_copy(out_sb[:, C:2 * C], ps_fs[:])
            nc.sync.dma_start(d_out, out_sb[:])

    nc.compile()
    return nc


def _host_inputs(x, lengths, w1, b1, w2, b2):
    x = np.asarray(x, np.float32)
    lengths = np.asarray(lengths)
    w1 = np.asarray(w1, np.float32)
    b1 = np.asarray(b1, np.float32)
    w2 = np.asarray(w2, np.float32)
    b2 = np.asarray(b2, np.float32)

    cstf0 = np.zeros((128, NF), np.float32)
    cstf0[:, CF_B1] = b1
    cstf0[:, CF_EPS] = EPSB
    sutri16 = np.triu(np.ones((16, 16), np.float32), 1)
    cstf0[0:16, CF_SUTRI16:CF_SUTRI16 + 16] = sutri16
    cstf0[0:16, CF_LTRI16:CF_LTRI16 + 16] = np.tril(
        np.ones((16, 16), np.float32), -1)
    cstf0[:, CF_ONESC] = 1.0

    cstr = np.zeros((128, NR), np.float32)
    cstr[:, CR_TRIL:CR_TRIL + 128] = np.tril(np.ones((128, 128), np.float32))
    cstr[:, CR_ONESC] = 1.0

    cstb = np.zeros((128, NB), np.float32)
    cstb[:, CB_TRIU:CB_TRIU + 128] = np.triu(np.ones((128, 128), np.float32))
    onescols = np.zeros((128, 256), np.float32)
    for i in range(NCH):
        onescols[:, 16 * i + i] = 1.0
    cstb[:, CB_ONESCOLS:CB_ONESCOLS + 256] = onescols
    sutri48 = np.zeros((48, 16), np.float32)
    sutri48[0:16] = sutri16
    sutri48[32:48] = sutri16
    cstb[0:48, CB_SUTRI48:CB_SUTRI48 + 16] = sutri48
    cstb[0:8, CB_SEL8:CB_SEL8 + 16] = np.eye(8, 16, dtype=np.float32)
    cstb[:, CB_W2] = w2[:, 0]
    cstb[:, CB_ONESC] = 1.0
    cstb = cstb.astype(BF)

    tt = np.arange(T)
    w1b = np.ascontiguousarray(
        w1.reshape(12, 128, A).transpose(1, 0, 2).reshape(128, 12 * A)
    ).astype(BF)

    maps = []
    for b in range(B):
        L = int(lengths[b])
        rcnt = (1.0 / np.minimum(tt + 1, max(L, 1))).astype(np.float32)
        maskexp = (float(b2[0]) +
                   np.where(tt < L, 0.0, NEG)).astype(np.float32)
        finalw = np.where(tt < L, 1.0 / max(L, 1), 0.0).astype(np.float32)
        cstf = cstf0.copy()
        cstf[:, CF_RCNT:CF_RCNT + 16] = rcnt.reshape(NCH, 128).T
        cstf[:, CF_MASKEXP:CF_MASKEXP + 16] = maskexp.reshape(NCH, 128).T
        cstr_b = cstr.copy()
        cstr_b[:, CR_FINALW:CR_FINALW + 16] = finalw.reshape(NCH, 128).T
        maps.append({
            "xT": np.ascontiguousarray(x[b].T).astype(BF),
            "x2T": np.ascontiguousarray((x[b] ** 2).T).astype(BF),
            "xN": np.ascontiguousarray(x[b]).astype(BF),
            "w1b": w1b,
            "cstf": cstf,
            "cstr": cstr_b,
            "cstb": cstb,
            "rcntb": np.ascontiguousarray(
                np.broadcast_to(rcnt[None, :], (128, T))).astype(BF),
        })
    return maps


def kernel(x, lengths, w1, b1, w2, b2):
    from concourse.bass_utils import run_bass_kernel_spmd

    if "nc" not in _CACHE:
        _CACHE["nc"] = _build()
    nc = _CACHE["nc"]
    maps = _host_inputs(x, lengths, w1, b1, w2, b2)
    res = run_bass_kernel_spmd(nc, maps, list(range(B))).results
    out = np.stack([res[b]["out"][0] for b in range(B)], axis=0)
    return out.astype(np.float32)
